# revision 35
# baseline (speedup 1.0000x reference)
"""Trainium2 Bass kernel for nn_FactorizedCrossAttention.

Key algebraic facts used (verified against the reference in fp64):
  * The "spatial" and "temporal" branches compute IDENTICAL per-position
    values: cross-attention over text tokens is independent per query row,
    and qt rows equal qs rows (same x row through the same Wq).  Hence
    spatial == temporal exactly.
  * concat([A, A]) @ Wst @ Wo == A @ ((Wst[:D] + Wst[D:]) @ Wo) — so both
    output projections fold into one 1024x1024 matrix Weff.
  * softmax scale (0.125) is folded into K on the host; the padding-mask
    bias is a per-token column vector applied inside the exp activation
    (ACT computes func(in + bias), bias indexed by partition = token), so
    masking is free and there is a single code path.  No max-subtraction:
    scores are O(1) so exp cannot overflow.

Softmax bookkeeping (vs the 590us baseline):
  * Per-head softmax sums are collected into ONE PSUM tile via one-hot
    selector matmuls accumulated across the 16 heads of a rowtile.  This
    kills the per-head [1,512] ln/exp ACT chain of the baseline (~175us
    of ACT busy + the serial dependency that stalled the PE); reciprocals
    are then one ln + one negated-exp ACT op per rowtile.
  * The reciprocal is broadcast to A^T's 128 partitions with ONE selector
    matmul per head-PAIR, halving the baseline's rank-1 broadcast count.
  * PV results (A^T unnormalized, one [128, 512] tile per head pair) are
    copied PSUM->SBUF fp32 right after the pair completes so PSUM stays
    within 8 banks; the tail does bcast-matmul + DVE multiply into the
    packed bf16 A^T.

PE efficiency (the big one, ~65us): every attention matmul (qk, PV,
sums, bcast) uses weights zero-padded to a full 128x128 block.  Partial
tile_size matmuls cannot pipeline back-to-back on the PE array and pay
the full ~219-cycle fill latency per instruction (~305-340ns observed);
full blocks issue at stream rate (~225ns).  The zero weight rows also
nullify garbage in the padded rhs partitions -- provided that garbage is
finite, hence the full-height exp (pad scores are exact zeros ->
exp(0)=1) and the one-time zeroing of the reciprocal pool buffers.
The previous rowtile's output projection is interleaved between heads in
4-matmul batches to cover ACT/DVE latencies.

Sharding: pure data-parallel over (B, T_frames): 32 frames / 8 cores =
4 frames (4096 query rows) per core; K/V/weights replicated.  No
collectives.

Device layout is "transposed activations": X^T, Q^T, A^T all live as
[feature-part, row-free] tiles so every matmul is a natural slice.  Head h
occupies partitions (h%2)*64..+64 of feature chunk h//2; K^T is replicated
on both partition halves so odd heads read lane-aligned operands, and odd
heads' PV output is placed at PSUM base 64 (tile_position) so A^T lands on
partitions 64..127 without any cross-partition copies.
"""

import sys

if "/opt/trn_rl_repo" not in sys.path:
    sys.path.insert(0, "/opt/trn_rl_repo")

from contextlib import ExitStack

import ml_dtypes
import numpy as np

import concourse.bass as bass
import concourse.mybir as mybir
import concourse.tile as tile
from concourse import bacc
from concourse.bass_utils import run_bass_kernel_spmd

BF16 = ml_dtypes.bfloat16

D = 1024           # d_model
H = 16             # num heads
G = 4              # query groups
HD = 64            # head dim
HPG = H // G       # heads per group
SCALE = 0.125
B, T, HW, TT = 2, 16, 1024, 77
NCORES = 8
FPC = (B * T) // NCORES      # frames per core = 4
ROWS = FPC * HW              # 4096 query rows per core
RT = 512                     # rows per row-tile
NRT = ROWS // RT             # 8
NK = D // 128                # 8 partition chunks of d_model

_PROG_CACHE = {}


def _patch_act_tables():
    """Force every activation onto the one table set that contains Exp, Ln
    and Copy together (natural_log_exp_and_others, same 400-interval
    precision).  Without this, bacc's table-load pass can alternate between
    table sets, costing a ~1.28us ACT_TABLE_LOAD per switch."""
    import concourse.bacc as _bm
    import concourse.hw_specs as _hw
    if getattr(_bm, "_act_tables_patched", False):
        return
    _orig = _hw.get_activation_tables

    def patched(arch):
        t = dict(_orig(arch))
        combo = None
        for name, funcs in t.items():
            if (mybir.ActivationFunctionType.Exp in funcs
                    and mybir.ActivationFunctionType.Ln in funcs):
                combo = name
                break
        if combo is not None:
            for name in list(t):
                if name != combo:
                    t[name] = set()
        return t

    _bm.get_activation_tables = patched
    _bm._act_tables_patched = True

# test.py can flip these for profiling runs
TRACE = False
TRACE_KWARGS = {}
LAST_RESULTS = None


def _build_program():
    _patch_act_tables()
    dt = mybir.dt
    nc = bacc.Bacc("TRN2", target_bir_lowering=False, debug=False,
                   num_devices=NCORES)

    xt = nc.dram_tensor("xt", [D, ROWS], dt.bfloat16, kind="ExternalInput").ap()
    wq = nc.dram_tensor("wq", [D, D], dt.bfloat16, kind="ExternalInput").ap()
    weff = nc.dram_tensor("weff", [D, D], dt.bfloat16, kind="ExternalInput").ap()
    # All attention weights are zero-padded to full 128x128 blocks: partial
    # tile_size matmuls (64/77/16-row contract) cannot pipeline back-to-back
    # on the PE and pay the full ~219-cycle array latency each; full blocks
    # issue at stream rate.  Zero weight rows also nullify whatever garbage
    # sits in the padded partitions of the rhs operands, so no explicit
    # zeroing is needed anywhere.
    # per-head qk blocks: [128, H*128], block h rows (h%2)*64..+64 = K_g^T
    kt = nc.dram_tensor("kt", [128, H * 128], dt.bfloat16, kind="ExternalInput").ap()
    # per-head PV blocks: [128, H*128], block h rows 0..76 = V_g at cols (h%2)*64..+64
    v64 = nc.dram_tensor("v64", [128, H * 128], dt.bfloat16, kind="ExternalInput").ap()
    # per-head one-hot sum selectors: [128, H*128], block h: col h = 1 on rows 0..76
    sel = nc.dram_tensor("sel", [128, H * 128], dt.bfloat16, kind="ExternalInput").ap()
    # per-pair broadcast selectors: [128, 8*128], pair c: row 2c -> cols 0..63,
    # row 2c+1 -> cols 64..127
    selp = nc.dram_tensor("selp", [128, 8 * 128], dt.bfloat16, kind="ExternalInput").ap()
    # padding-mask bias as a per-partition column vector, applied inside the
    # exp activation (func(scale*in + bias)); all-zero when the mask is all-True
    biasr = nc.dram_tensor("biasr", [128, 1], dt.float32, kind="ExternalInput").ap()
    out = nc.dram_tensor("out", [ROWS, D], dt.float32, kind="ExternalOutput").ap()

    with tile.TileContext(nc) as tc, ExitStack() as ctx:
        wpool = ctx.enter_context(tc.tile_pool(name="weights", bufs=1))
        xpool = ctx.enter_context(tc.tile_pool(name="xt", bufs=2))
        qpool = ctx.enter_context(tc.tile_pool(name="qt", bufs=2))
        apool = ctx.enter_context(tc.tile_pool(name="at", bufs=2))
        ppool = ctx.enter_context(tc.tile_pool(name="pt", bufs=4))
        aupool = ctx.enter_context(tc.tile_pool(name="au", bufs=9))
        rpool = ctx.enter_context(tc.tile_pool(name="recip", bufs=2))
        rbpool = ctx.enter_context(tc.tile_pool(name="recipb", bufs=2))
        opool = ctx.enter_context(tc.tile_pool(name="osb", bufs=3))
        # 8 PSUM banks: qp(2) + sprb(2, scores+bcast) + ap(2) + sums(1) + op(1)
        qpsum = ctx.enter_context(tc.tile_pool(name="qpsum", bufs=2, space="PSUM"))
        spsum = ctx.enter_context(tc.tile_pool(name="spsum", bufs=2, space="PSUM"))
        apsum = ctx.enter_context(tc.tile_pool(name="apsum", bufs=2, space="PSUM"))
        supsum = ctx.enter_context(tc.tile_pool(name="supsum", bufs=1, space="PSUM"))
        opsum = ctx.enter_context(tc.tile_pool(name="opsum", bufs=1, space="PSUM"))

        # --- resident weights; first rowtile's X^T + Wq lead the DMA queue so
        # the first Qproj chain starts ~8us earlier
        wq_t = wpool.tile([128, NK * D], dt.bfloat16, tag="wq")
        weff_t = wpool.tile([128, NK * D], dt.bfloat16, tag="weff")
        # startup is DMA-bound: spread the critical first loads (X^T rowtile 0
        # + Wq, 3MB total) across all three DMA trigger queues (SP, ACT,
        # GPSIMD) so they transfer in parallel; full-chunk shapes only (column
        # splits of these DMAs measured ~20% slower overall)
        xt_r = xt.rearrange("(kc p) r -> p kc r", p=128)
        xt0_t = xpool.tile([128, NK * RT], dt.bfloat16, tag="xt")
        nc.scalar.dma_start(
            out=xt0_t[:, :].rearrange("p (kc r) -> p kc r", r=RT),
            in_=xt_r[:, :, 0:RT])
        nc.sync.dma_start(
            out=wq_t[:, :].rearrange("p (kc c) -> p kc c", c=D),
            in_=wq.rearrange("(kc p) c -> p kc c", p=128))
        kt_t = wpool.tile([128, H * 128], dt.bfloat16, tag="kt")
        nc.sync.dma_start(out=kt_t[:], in_=kt[:, :])
        v64_t = wpool.tile([128, H * 128], dt.bfloat16, tag="v64")
        nc.sync.dma_start(out=v64_t[:], in_=v64[:, :])
        sel_t = wpool.tile([128, H * 128], dt.bfloat16, tag="sel")
        nc.sync.dma_start(out=sel_t[:], in_=sel[:, :])
        selp_t = wpool.tile([128, 8 * 128], dt.bfloat16, tag="selp")
        nc.sync.dma_start(out=selp_t[:], in_=selp[:, :])
        bias_t = wpool.tile([128, 1], dt.float32, tag="bias")
        nc.sync.dma_start(out=bias_t[:], in_=biasr[:, :])
        nc.sync.dma_start(
            out=weff_t[:, :].rearrange("p (kc c) -> p kc c", c=D),
            in_=weff.rearrange("(kc p) c -> p kc c", p=128))

        # one-time zeroing of the reciprocal tiles' pad rows: the tail's
        # broadcast matmul streams rcb[16:128] into zero weight rows, and
        # those bytes are never written by the [0:16] ACT ops, so clearing
        # the pool's physical buffers once keeps them finite forever
        for _ in range(2):
            rcz = rbpool.tile([128, RT], dt.bfloat16, tag="rcb")
            nc.vector.memset(rcz[:], 0.0)

        prev = None
        for rt in range(NRT):
            rsl = bass.ts(rt, RT)  # row slice in DRAM

            # --- load X^T row-tile: [1024 din, 512 rows] packed [128, 8*512]
            if rt == 0:
                xt_t = xt0_t
            else:
                xt_t = xpool.tile([128, NK * RT], dt.bfloat16, tag="xt")
                nc.sync.dma_start(
                    out=xt_t[:, :].rearrange("p (kc r) -> p kc r", r=RT),
                    in_=xt_r[:, :, rt * RT:(rt + 1) * RT],
                )

            # --- Q^T = Wq^T @ X^T : [1024 qcols, 512 rows] packed [128, 8*512]
            qt_t = qpool.tile([128, NK * RT], dt.bfloat16, tag="qt")
            for cc in range(NK):
                qp = qpsum.tile([128, RT], dt.float32, tag="qp")
                for kc in range(NK):
                    nc.tensor.matmul(
                        qp[:],
                        lhsT=wq_t[:, kc * D + cc * 128: kc * D + (cc + 1) * 128],
                        rhs=xt_t[:, kc * RT:(kc + 1) * RT],
                        start=(kc == 0), stop=(kc == NK - 1),
                    )
                nc.vector.tensor_copy(qt_t[:, cc * RT:(cc + 1) * RT], qp[:])

            # --- attention per head -> A^T packed [128, 8*512] (bf16)
            at_t = apool.tile([128, NK * RT], dt.bfloat16, tag="at")

            def wproj_gen(prt, pat):
                # previous rowtile's Out projection as a stream of small
                # PE batches, pulled between attention heads so the PE has
                # dense independent work during ACT/DVE latencies.  PSUM
                # double-buffers by alternating the op pool with the (idle
                # during the head loop) Qproj pool.
                for rc4 in range(4):
                    ot = opool.tile([128, D], dt.float32, tag="ot")
                    for oc in range(2):
                        pool = opsum if (rc4 * 2 + oc) % 2 == 0 else qpsum
                        op_ = pool.tile([128, RT], dt.float32,
                                        tag="op" if pool is opsum else "qp")
                        for ac in range(NK):
                            nc.tensor.matmul(
                                op_[:],
                                lhsT=pat[:, ac * RT + rc4 * 128: ac * RT + (rc4 + 1) * 128],
                                rhs=weff_t[:, ac * D + oc * 512: ac * D + (oc + 1) * 512],
                                start=(ac == 0), stop=(ac == NK - 1),
                            )
                            if ac % 4 == 3:
                                yield
                        nc.vector.tensor_copy(ot[:, oc * 512:(oc + 1) * 512], op_[:])
                    nc.sync.dma_start(
                        out=out[prt * RT + rc4 * 128: prt * RT + (rc4 + 1) * 128, :],
                        in_=ot[:],
                    )

            wops = wproj_gen(prev[0], prev[1]) if prev is not None else None

            _done = object()

            def drain(k):
                if wops is None:
                    return
                for _ in range(k):
                    if next(wops, _done) is _done:
                        break

            sums = supsum.tile([128, RT], dt.float32, tag="sums")
            au_list = []
            pair_ap = [None]

            def head_front(h):
                csl = bass.ds((h // 2) * RT, RT)
                sp = spsum.tile([128, RT], dt.float32, tag="sprb")
                nc.tensor.matmul(
                    sp[:],
                    lhsT=kt_t[:, h * 128:(h + 1) * 128],
                    rhs=qt_t[:, csl],
                    start=True, stop=True,
                )
                # full-height exp: pad score rows are exact zeros from the
                # zero-padded qk weights, so pt rows 77..127 become exp(0)=1,
                # which the zero-padded sel/vp weight rows then ignore (a
                # finite value is required: 0-weight x inf/NaN garbage = NaN)
                pt = ppool.tile([128, RT], dt.bfloat16, tag="pt")
                nc.scalar.activation(pt[:], sp[:],
                                     mybir.ActivationFunctionType.Exp,
                                     bias=bias_t[:])
                return pt

            def head_back(h, pt):
                po = (h % 2) * HD
                # head h's softmax sum -> row h of the shared [128, RT] tile
                # (rows 16..127 accumulate zeros from the padded selector)
                nc.tensor.matmul(
                    sums[:],
                    lhsT=sel_t[:, h * 128:(h + 1) * 128],
                    rhs=pt[:],
                    start=(h == 0), stop=(h == H - 1),
                )
                if po == 0:
                    ap_ = apsum.tile([128, RT], dt.float32, tag="ap")
                    pair_ap[0] = ap_
                nc.tensor.matmul(
                    pair_ap[0][:],
                    lhsT=v64_t[:, h * 128:(h + 1) * 128],
                    rhs=pt[:],
                    start=(po == 0), stop=(po != 0),
                )
                if po != 0:
                    # pair complete: stash unnormalized A^T pair in SBUF fp32
                    au = aupool.tile([128, RT], dt.float32, tag="au")
                    nc.vector.tensor_copy(au[:], pair_ap[0][:])
                    au_list.append(au)

            # software-pipelined head loop: exp(h) overlaps qk(h+1) and the
            # previous head's sums/PV plus a Wproj batch
            prev_pt = None
            for h in range(H):
                pt = head_front(h)
                if prev_pt is not None:
                    head_back(h - 1, prev_pt)
                prev_pt = pt
                if h >= 2:
                    drain(1)
            head_back(H - 1, prev_pt)

            # --- tail: one reciprocal for all 16 heads, then per-pair
            # broadcast + normalize into packed A^T
            rcf = rpool.tile([16, RT], dt.float32, tag="rcf")
            nc.scalar.activation(rcf[0:16, :], sums[0:16, :],
                                 mybir.ActivationFunctionType.Ln)
            rcb = rbpool.tile([128, RT], dt.bfloat16, tag="rcb")
            nc.scalar.activation(rcb[0:16, :], rcf[0:16, :],
                                 mybir.ActivationFunctionType.Exp,
                                 scale=-1.0)
            drain(2)
            for c in range(NK):
                rb = spsum.tile([128, RT], dt.float32, tag="sprb")
                nc.tensor.matmul(
                    rb[:],
                    lhsT=selp_t[:, c * 128:(c + 1) * 128],
                    rhs=rcb[:],
                    start=True, stop=True,
                )
                nc.vector.tensor_mul(
                    at_t[:, c * RT:(c + 1) * RT], au_list[c][:], rb[:],
                )
                drain(1)
            drain(99)

            prev = (rt, at_t)

        # drain: Wproj of the final rowtile
        for _ in wproj_gen(prev[0], prev[1]):
            pass

    nc.compile()
    return nc


def _get_program():
    if "p" not in _PROG_CACHE:
        _PROG_CACHE["p"] = _build_program()
    return _PROG_CACHE["p"]


def _prep_inputs(x, te, mask, Wq, Wk, Wv, Wo, Wst):
    """Host-side fp32 weight prep + per-core shard maps."""
    K = (te @ Wk).reshape(B, TT, G, HD) * SCALE
    V = (te @ Wv).reshape(B, TT, G, HD)
    Weff = ((Wst[:D] + Wst[D:]) @ Wo).astype(np.float32)

    wq_b = Wq.astype(BF16)
    weff_b = Weff.astype(BF16)

    # per-head one-hot sum selectors [128, H*128]: block h col h = 1 on rows 0..76
    sel_np = np.zeros((128, H * 128), np.float32)
    for h in range(H):
        sel_np[0:TT, h * 128 + h] = 1.0
    sel_b = sel_np.astype(BF16)
    # per-pair broadcast selectors [128, 8*128]
    selp_np = np.zeros((128, 8 * 128), np.float32)
    for c in range(8):
        selp_np[2 * c, c * 128: c * 128 + 64] = 1.0
        selp_np[2 * c + 1, c * 128 + 64: c * 128 + 128] = 1.0
    selp_b = selp_np.astype(BF16)

    kt_b, v_b, bias_b = [], [], []
    for b in range(B):
        # qk blocks [128, H*128]: block h rows (h%2)*64..+64, cols 0..76 = K_g^T
        ktq = np.zeros((128, H * 128), np.float32)
        # PV blocks [128, H*128]: block h rows 0..76, cols (h%2)*64..+64 = V_g
        vp = np.zeros((128, H * 128), np.float32)
        for h in range(H):
            g = h // HPG
            po = (h % 2) * HD
            ktq[po:po + HD, h * 128:h * 128 + TT] = K[b, :, g, :].T
            vp[0:TT, h * 128 + po:h * 128 + po + HD] = V[b, :, g, :]
        kt_b.append(ktq.astype(BF16))
        v_b.append(vp.astype(BF16))
        bv = np.zeros((128, 1), np.float32)
        bv[0:TT, 0] = np.where(mask[b], 0.0, -30.0)
        bias_b.append(bv)

    in_maps = []
    for c in range(NCORES):
        b = c // (NCORES // B)
        fr = (c % (NCORES // B)) * FPC
        xc = x[b, fr:fr + FPC].reshape(ROWS, D).astype(BF16)
        m = {
            "xt": np.ascontiguousarray(xc.T),
            "wq": wq_b,
            "weff": weff_b,
            "kt": kt_b[b],
            "v64": v_b[b],
            "sel": sel_b,
            "selp": selp_b,
            "biasr": bias_b[b],
        }
        in_maps.append(m)
    return in_maps


def kernel(x, text_embeddings, padding_mask, use_mqa=0, use_qk_norm=0,
           Wq=None, Wk=None, Wv=None, Wo=None, Wst=None):
    global LAST_RESULTS
    x = np.asarray(x, np.float32)
    te = np.asarray(text_embeddings, np.float32)
    mask = np.asarray(padding_mask).astype(bool)
    Wq = np.asarray(Wq, np.float32)
    Wk = np.asarray(Wk, np.float32)
    Wv = np.asarray(Wv, np.float32)
    Wo = np.asarray(Wo, np.float32)
    Wst = np.asarray(Wst, np.float32)
    assert x.shape == (B, T, HW, D) and te.shape == (B, TT, D)

    in_maps = _prep_inputs(x, te, mask, Wq, Wk, Wv, Wo, Wst)
    nc = _get_program()

    res = run_bass_kernel_spmd(nc, in_maps, list(range(NCORES)),
                               trace=TRACE, **TRACE_KWARGS)
    LAST_RESULTS = res

    outp = np.empty((B, T, HW, D), np.float32)
    for c in range(NCORES):
        b = c // (NCORES // B)
        fr = (c % (NCORES // B)) * FPC
        outp[b, fr:fr + FPC] = res.results[c]["out"].reshape(FPC, HW, D)
    return outp


# revision 36
# speedup vs baseline: 1.0144x; 1.0144x over previous
"""Trainium2 Bass kernel for nn_FactorizedCrossAttention.

Key algebraic facts used (verified against the reference in fp64):
  * The "spatial" and "temporal" branches compute IDENTICAL per-position
    values: cross-attention over text tokens is independent per query row,
    and qt rows equal qs rows (same x row through the same Wq).  Hence
    spatial == temporal exactly.
  * concat([A, A]) @ Wst @ Wo == A @ ((Wst[:D] + Wst[D:]) @ Wo) — so both
    output projections fold into one 1024x1024 matrix Weff.
  * softmax scale (0.125) is folded into K on the host; the padding-mask
    bias is a per-token column vector applied inside the exp activation
    (ACT computes func(in + bias), bias indexed by partition = token), so
    masking is free and there is a single code path.  No max-subtraction:
    scores are O(1) so exp cannot overflow.

Softmax bookkeeping (vs the 590us baseline):
  * Per-head softmax sums are collected into ONE PSUM tile via one-hot
    selector matmuls accumulated across the 16 heads of a rowtile.  This
    kills the per-head [1,512] ln/exp ACT chain of the baseline (~175us
    of ACT busy + the serial dependency that stalled the PE); reciprocals
    are then one ln + one negated-exp ACT op per rowtile.
  * The reciprocal is broadcast to A^T's 128 partitions with ONE selector
    matmul per head-PAIR, halving the baseline's rank-1 broadcast count.
  * PV results (A^T unnormalized, one [128, 512] tile per head pair) are
    copied PSUM->SBUF fp32 right after the pair completes so PSUM stays
    within 8 banks; the tail does bcast-matmul + DVE multiply into the
    packed bf16 A^T.

PE efficiency (the big one, ~65us): every attention matmul (qk, PV,
sums, bcast) uses weights zero-padded to a full 128x128 block.  Partial
tile_size matmuls cannot pipeline back-to-back on the PE array and pay
the full ~219-cycle fill latency per instruction (~305-340ns observed);
full blocks issue at stream rate (~225ns).  The zero weight rows also
nullify garbage in the padded rhs partitions -- provided that garbage is
finite, hence the full-height exp (pad scores are exact zeros ->
exp(0)=1) and the one-time zeroing of the reciprocal pool buffers.
The previous rowtile's output projection is interleaved between heads in
4-matmul batches to cover ACT/DVE latencies.

Sharding: pure data-parallel over (B, T_frames): 32 frames / 8 cores =
4 frames (4096 query rows) per core; K/V/weights replicated.  No
collectives.

Device layout is "transposed activations": X^T, Q^T, A^T all live as
[feature-part, row-free] tiles so every matmul is a natural slice.  Head h
occupies partitions (h%2)*64..+64 of feature chunk h//2; K^T is replicated
on both partition halves so odd heads read lane-aligned operands, and odd
heads' PV output is placed at PSUM base 64 (tile_position) so A^T lands on
partitions 64..127 without any cross-partition copies.
"""

import sys

if "/opt/trn_rl_repo" not in sys.path:
    sys.path.insert(0, "/opt/trn_rl_repo")

from contextlib import ExitStack

import ml_dtypes
import numpy as np

import concourse.bass as bass
import concourse.mybir as mybir
import concourse.tile as tile
from concourse import bacc
from concourse.bass_utils import run_bass_kernel_spmd

BF16 = ml_dtypes.bfloat16

D = 1024           # d_model
H = 16             # num heads
G = 4              # query groups
HD = 64            # head dim
HPG = H // G       # heads per group
SCALE = 0.125
B, T, HW, TT = 2, 16, 1024, 77
NCORES = 8
FPC = (B * T) // NCORES      # frames per core = 4
ROWS = FPC * HW              # 4096 query rows per core
RT = 512                     # rows per row-tile
NRT = ROWS // RT             # 8
NK = D // 128                # 8 partition chunks of d_model

_PROG_CACHE = {}


def _patch_act_tables():
    """Force every activation onto the one table set that contains Exp, Ln
    and Copy together (natural_log_exp_and_others, same 400-interval
    precision).  Without this, bacc's table-load pass can alternate between
    table sets, costing a ~1.28us ACT_TABLE_LOAD per switch."""
    import concourse.bacc as _bm
    import concourse.hw_specs as _hw
    if getattr(_bm, "_act_tables_patched", False):
        return
    _orig = _hw.get_activation_tables

    def patched(arch):
        t = dict(_orig(arch))
        combo = None
        for name, funcs in t.items():
            if (mybir.ActivationFunctionType.Exp in funcs
                    and mybir.ActivationFunctionType.Ln in funcs):
                combo = name
                break
        if combo is not None:
            for name in list(t):
                if name != combo:
                    t[name] = set()
        return t

    _bm.get_activation_tables = patched
    _bm._act_tables_patched = True

# test.py can flip these for profiling runs
TRACE = False
TRACE_KWARGS = {}
LAST_RESULTS = None


def _build_program():
    _patch_act_tables()
    dt = mybir.dt
    nc = bacc.Bacc("TRN2", target_bir_lowering=False, debug=False,
                   num_devices=NCORES)

    xt = nc.dram_tensor("xt", [D, ROWS], dt.bfloat16, kind="ExternalInput").ap()
    wq = nc.dram_tensor("wq", [D, D], dt.bfloat16, kind="ExternalInput").ap()
    weff = nc.dram_tensor("weff", [D, D], dt.bfloat16, kind="ExternalInput").ap()
    # All attention weights are zero-padded to full 128x128 blocks: partial
    # tile_size matmuls (64/77/16-row contract) cannot pipeline back-to-back
    # on the PE and pay the full ~219-cycle array latency each; full blocks
    # issue at stream rate.  Zero weight rows also nullify whatever garbage
    # sits in the padded partitions of the rhs operands, so no explicit
    # zeroing is needed anywhere.
    # per-head qk blocks: [128, H*128], block h rows (h%2)*64..+64 = K_g^T
    kt = nc.dram_tensor("kt", [128, H * 128], dt.bfloat16, kind="ExternalInput").ap()
    # per-head PV blocks: [128, H*128], block h rows 0..76 = V_g at cols (h%2)*64..+64
    v64 = nc.dram_tensor("v64", [128, H * 128], dt.bfloat16, kind="ExternalInput").ap()
    # per-head one-hot sum selectors: [128, H*128], block h: col h = 1 on rows 0..76
    sel = nc.dram_tensor("sel", [128, H * 128], dt.bfloat16, kind="ExternalInput").ap()
    # per-pair broadcast selectors: [128, 8*128], pair c: row 2c -> cols 0..63,
    # row 2c+1 -> cols 64..127
    selp = nc.dram_tensor("selp", [128, 8 * 128], dt.bfloat16, kind="ExternalInput").ap()
    # padding-mask bias as a per-partition column vector, applied inside the
    # exp activation (func(scale*in + bias)); all-zero when the mask is all-True
    biasr = nc.dram_tensor("biasr", [128, 1], dt.float32, kind="ExternalInput").ap()
    out = nc.dram_tensor("out", [ROWS, D], dt.float32, kind="ExternalOutput").ap()

    with tile.TileContext(nc) as tc, ExitStack() as ctx:
        wpool = ctx.enter_context(tc.tile_pool(name="weights", bufs=1))
        xpool = ctx.enter_context(tc.tile_pool(name="xt", bufs=2))
        qpool = ctx.enter_context(tc.tile_pool(name="qt", bufs=2))
        apool = ctx.enter_context(tc.tile_pool(name="at", bufs=2))
        ppool = ctx.enter_context(tc.tile_pool(name="pt", bufs=4))
        aupool = ctx.enter_context(tc.tile_pool(name="au", bufs=9))
        rpool = ctx.enter_context(tc.tile_pool(name="recip", bufs=2))
        rbpool = ctx.enter_context(tc.tile_pool(name="recipb", bufs=2))
        opool = ctx.enter_context(tc.tile_pool(name="osb", bufs=3))
        # 8 PSUM banks: qp(2) + sprb(2, scores+bcast) + ap(2) + sums(1) + op(1)
        qpsum = ctx.enter_context(tc.tile_pool(name="qpsum", bufs=2, space="PSUM"))
        spsum = ctx.enter_context(tc.tile_pool(name="spsum", bufs=2, space="PSUM"))
        apsum = ctx.enter_context(tc.tile_pool(name="apsum", bufs=2, space="PSUM"))
        supsum = ctx.enter_context(tc.tile_pool(name="supsum", bufs=1, space="PSUM"))
        opsum = ctx.enter_context(tc.tile_pool(name="opsum", bufs=1, space="PSUM"))

        # --- resident weights; first rowtile's X^T + Wq lead the DMA queue so
        # the first Qproj chain starts ~8us earlier
        wq_t = wpool.tile([128, NK * D], dt.bfloat16, tag="wq")
        weff_t = wpool.tile([128, NK * D], dt.bfloat16, tag="weff")
        # startup is DMA-bound: spread the critical first loads (X^T rowtile 0
        # + Wq, 3MB total) across all three DMA trigger queues (SP, ACT,
        # GPSIMD) so they transfer in parallel; full-chunk shapes only (column
        # splits of these DMAs measured ~20% slower overall)
        # chunked startup DMAs spread across engines (a single merged DMA
        # runs on one engine and takes ~2x longer to land); merged 3D-AP
        # DMAs are used for the non-latency-critical loads below
        xt_r = xt.rearrange("(kc p) r -> p kc r", p=128)
        xt0_t = xpool.tile([128, NK * RT], dt.bfloat16, tag="xt")
        for kc in range(NK):
            nc.scalar.dma_start(out=xt0_t[:, kc * RT:(kc + 1) * RT],
                                in_=xt[kc * 128:(kc + 1) * 128, 0:RT])
            nc.sync.dma_start(out=wq_t[:, kc * D:(kc + 1) * D],
                              in_=wq[kc * 128:(kc + 1) * 128, :])
        kt_t = wpool.tile([128, H * 128], dt.bfloat16, tag="kt")
        nc.sync.dma_start(out=kt_t[:], in_=kt[:, :])
        v64_t = wpool.tile([128, H * 128], dt.bfloat16, tag="v64")
        nc.sync.dma_start(out=v64_t[:], in_=v64[:, :])
        sel_t = wpool.tile([128, H * 128], dt.bfloat16, tag="sel")
        nc.sync.dma_start(out=sel_t[:], in_=sel[:, :])
        selp_t = wpool.tile([128, 8 * 128], dt.bfloat16, tag="selp")
        nc.sync.dma_start(out=selp_t[:], in_=selp[:, :])
        bias_t = wpool.tile([128, 1], dt.float32, tag="bias")
        nc.sync.dma_start(out=bias_t[:], in_=biasr[:, :])
        nc.sync.dma_start(
            out=weff_t[:, :].rearrange("p (kc c) -> p kc c", c=D),
            in_=weff.rearrange("(kc p) c -> p kc c", p=128))

        # one-time zeroing of the reciprocal tiles' pad rows: the tail's
        # broadcast matmul streams rcb[16:128] into zero weight rows, and
        # those bytes are never written by the [0:16] ACT ops, so clearing
        # the pool's physical buffers once keeps them finite forever
        for _ in range(2):
            rcz = rbpool.tile([128, RT], dt.bfloat16, tag="rcb")
            nc.vector.memset(rcz[:], 0.0)

        prev = None
        for rt in range(NRT):
            rsl = bass.ts(rt, RT)  # row slice in DRAM

            # --- load X^T row-tile: [1024 din, 512 rows] packed [128, 8*512]
            if rt == 0:
                xt_t = xt0_t
            else:
                xt_t = xpool.tile([128, NK * RT], dt.bfloat16, tag="xt")
                nc.sync.dma_start(
                    out=xt_t[:, :].rearrange("p (kc r) -> p kc r", r=RT),
                    in_=xt_r[:, :, rt * RT:(rt + 1) * RT],
                )

            # --- Q^T = Wq^T @ X^T : [1024 qcols, 512 rows] packed [128, 8*512]
            qt_t = qpool.tile([128, NK * RT], dt.bfloat16, tag="qt")
            for cc in range(NK):
                qp = qpsum.tile([128, RT], dt.float32, tag="qp")
                for kc in range(NK):
                    nc.tensor.matmul(
                        qp[:],
                        lhsT=wq_t[:, kc * D + cc * 128: kc * D + (cc + 1) * 128],
                        rhs=xt_t[:, kc * RT:(kc + 1) * RT],
                        start=(kc == 0), stop=(kc == NK - 1),
                    )
                nc.vector.tensor_copy(qt_t[:, cc * RT:(cc + 1) * RT], qp[:])

            # --- attention per head -> A^T packed [128, 8*512] (bf16)
            at_t = apool.tile([128, NK * RT], dt.bfloat16, tag="at")

            def wproj_gen(prt, pat):
                # previous rowtile's Out projection as a stream of small
                # PE batches, pulled between attention heads so the PE has
                # dense independent work during ACT/DVE latencies.  PSUM
                # double-buffers by alternating the op pool with the (idle
                # during the head loop) Qproj pool.
                for rc4 in range(4):
                    ot = opool.tile([128, D], dt.float32, tag="ot")
                    for oc in range(2):
                        pool = opsum if (rc4 * 2 + oc) % 2 == 0 else qpsum
                        op_ = pool.tile([128, RT], dt.float32,
                                        tag="op" if pool is opsum else "qp")
                        for ac in range(NK):
                            nc.tensor.matmul(
                                op_[:],
                                lhsT=pat[:, ac * RT + rc4 * 128: ac * RT + (rc4 + 1) * 128],
                                rhs=weff_t[:, ac * D + oc * 512: ac * D + (oc + 1) * 512],
                                start=(ac == 0), stop=(ac == NK - 1),
                            )
                            if ac % 4 == 3:
                                yield
                        nc.vector.tensor_copy(ot[:, oc * 512:(oc + 1) * 512], op_[:])
                    nc.sync.dma_start(
                        out=out[prt * RT + rc4 * 128: prt * RT + (rc4 + 1) * 128, :],
                        in_=ot[:],
                    )

            wops = wproj_gen(prev[0], prev[1]) if prev is not None else None

            _done = object()

            def drain(k):
                if wops is None:
                    return
                for _ in range(k):
                    if next(wops, _done) is _done:
                        break

            sums = supsum.tile([128, RT], dt.float32, tag="sums")
            au_list = []
            pair_ap = [None]

            def head_front(h):
                csl = bass.ds((h // 2) * RT, RT)
                sp = spsum.tile([128, RT], dt.float32, tag="sprb")
                nc.tensor.matmul(
                    sp[:],
                    lhsT=kt_t[:, h * 128:(h + 1) * 128],
                    rhs=qt_t[:, csl],
                    start=True, stop=True,
                )
                # full-height exp: pad score rows are exact zeros from the
                # zero-padded qk weights, so pt rows 77..127 become exp(0)=1,
                # which the zero-padded sel/vp weight rows then ignore (a
                # finite value is required: 0-weight x inf/NaN garbage = NaN)
                pt = ppool.tile([128, RT], dt.bfloat16, tag="pt")
                nc.scalar.activation(pt[:], sp[:],
                                     mybir.ActivationFunctionType.Exp,
                                     bias=bias_t[:])
                return pt

            def head_back(h, pt):
                po = (h % 2) * HD
                # head h's softmax sum -> row h of the shared [128, RT] tile
                # (rows 16..127 accumulate zeros from the padded selector)
                nc.tensor.matmul(
                    sums[:],
                    lhsT=sel_t[:, h * 128:(h + 1) * 128],
                    rhs=pt[:],
                    start=(h == 0), stop=(h == H - 1),
                )
                if po == 0:
                    ap_ = apsum.tile([128, RT], dt.float32, tag="ap")
                    pair_ap[0] = ap_
                nc.tensor.matmul(
                    pair_ap[0][:],
                    lhsT=v64_t[:, h * 128:(h + 1) * 128],
                    rhs=pt[:],
                    start=(po == 0), stop=(po != 0),
                )
                if po != 0:
                    # pair complete: stash unnormalized A^T pair in SBUF fp32
                    au = aupool.tile([128, RT], dt.float32, tag="au")
                    nc.vector.tensor_copy(au[:], pair_ap[0][:])
                    au_list.append(au)

            # software-pipelined head loop: exp(h) overlaps qk(h+1) and the
            # previous head's sums/PV plus a Wproj batch
            prev_pt = None
            for h in range(H):
                pt = head_front(h)
                if prev_pt is not None:
                    head_back(h - 1, prev_pt)
                prev_pt = pt
                if h >= 2:
                    drain(1)
            head_back(H - 1, prev_pt)

            # --- tail: one reciprocal for all 16 heads, then per-pair
            # broadcast + normalize into packed A^T
            rcf = rpool.tile([16, RT], dt.float32, tag="rcf")
            nc.scalar.activation(rcf[0:16, :], sums[0:16, :],
                                 mybir.ActivationFunctionType.Ln)
            rcb = rbpool.tile([128, RT], dt.bfloat16, tag="rcb")
            nc.scalar.activation(rcb[0:16, :], rcf[0:16, :],
                                 mybir.ActivationFunctionType.Exp,
                                 scale=-1.0)
            drain(2)
            for c in range(NK):
                rb = spsum.tile([128, RT], dt.float32, tag="sprb")
                nc.tensor.matmul(
                    rb[:],
                    lhsT=selp_t[:, c * 128:(c + 1) * 128],
                    rhs=rcb[:],
                    start=True, stop=True,
                )
                nc.vector.tensor_mul(
                    at_t[:, c * RT:(c + 1) * RT], au_list[c][:], rb[:],
                )
                drain(1)
            drain(99)

            prev = (rt, at_t)

        # drain: Wproj of the final rowtile
        for _ in wproj_gen(prev[0], prev[1]):
            pass

    nc.compile()
    return nc


def _get_program():
    if "p" not in _PROG_CACHE:
        _PROG_CACHE["p"] = _build_program()
    return _PROG_CACHE["p"]


def _prep_inputs(x, te, mask, Wq, Wk, Wv, Wo, Wst):
    """Host-side fp32 weight prep + per-core shard maps."""
    K = (te @ Wk).reshape(B, TT, G, HD) * SCALE
    V = (te @ Wv).reshape(B, TT, G, HD)
    Weff = ((Wst[:D] + Wst[D:]) @ Wo).astype(np.float32)

    wq_b = Wq.astype(BF16)
    weff_b = Weff.astype(BF16)

    # per-head one-hot sum selectors [128, H*128]: block h col h = 1 on rows 0..76
    sel_np = np.zeros((128, H * 128), np.float32)
    for h in range(H):
        sel_np[0:TT, h * 128 + h] = 1.0
    sel_b = sel_np.astype(BF16)
    # per-pair broadcast selectors [128, 8*128]
    selp_np = np.zeros((128, 8 * 128), np.float32)
    for c in range(8):
        selp_np[2 * c, c * 128: c * 128 + 64] = 1.0
        selp_np[2 * c + 1, c * 128 + 64: c * 128 + 128] = 1.0
    selp_b = selp_np.astype(BF16)

    kt_b, v_b, bias_b = [], [], []
    for b in range(B):
        # qk blocks [128, H*128]: block h rows (h%2)*64..+64, cols 0..76 = K_g^T
        ktq = np.zeros((128, H * 128), np.float32)
        # PV blocks [128, H*128]: block h rows 0..76, cols (h%2)*64..+64 = V_g
        vp = np.zeros((128, H * 128), np.float32)
        for h in range(H):
            g = h // HPG
            po = (h % 2) * HD
            ktq[po:po + HD, h * 128:h * 128 + TT] = K[b, :, g, :].T
            vp[0:TT, h * 128 + po:h * 128 + po + HD] = V[b, :, g, :]
        kt_b.append(ktq.astype(BF16))
        v_b.append(vp.astype(BF16))
        bv = np.zeros((128, 1), np.float32)
        bv[0:TT, 0] = np.where(mask[b], 0.0, -30.0)
        bias_b.append(bv)

    in_maps = []
    for c in range(NCORES):
        b = c // (NCORES // B)
        fr = (c % (NCORES // B)) * FPC
        xc = x[b, fr:fr + FPC].reshape(ROWS, D).astype(BF16)
        m = {
            "xt": np.ascontiguousarray(xc.T),
            "wq": wq_b,
            "weff": weff_b,
            "kt": kt_b[b],
            "v64": v_b[b],
            "sel": sel_b,
            "selp": selp_b,
            "biasr": bias_b[b],
        }
        in_maps.append(m)
    return in_maps


def kernel(x, text_embeddings, padding_mask, use_mqa=0, use_qk_norm=0,
           Wq=None, Wk=None, Wv=None, Wo=None, Wst=None):
    global LAST_RESULTS
    x = np.asarray(x, np.float32)
    te = np.asarray(text_embeddings, np.float32)
    mask = np.asarray(padding_mask).astype(bool)
    Wq = np.asarray(Wq, np.float32)
    Wk = np.asarray(Wk, np.float32)
    Wv = np.asarray(Wv, np.float32)
    Wo = np.asarray(Wo, np.float32)
    Wst = np.asarray(Wst, np.float32)
    assert x.shape == (B, T, HW, D) and te.shape == (B, TT, D)

    in_maps = _prep_inputs(x, te, mask, Wq, Wk, Wv, Wo, Wst)
    nc = _get_program()

    res = run_bass_kernel_spmd(nc, in_maps, list(range(NCORES)),
                               trace=TRACE, **TRACE_KWARGS)
    LAST_RESULTS = res

    outp = np.empty((B, T, HW, D), np.float32)
    for c in range(NCORES):
        b = c // (NCORES // B)
        fr = (c % (NCORES // B)) * FPC
        outp[b, fr:fr + FPC] = res.results[c]["out"].reshape(FPC, HW, D)
    return outp


# revision 41
# speedup vs baseline: 1.0203x; 1.0058x over previous
"""Trainium2 Bass kernel for nn_FactorizedCrossAttention.

Key algebraic facts used (verified against the reference in fp64):
  * The "spatial" and "temporal" branches compute IDENTICAL per-position
    values: cross-attention over text tokens is independent per query row,
    and qt rows equal qs rows (same x row through the same Wq).  Hence
    spatial == temporal exactly.
  * concat([A, A]) @ Wst @ Wo == A @ ((Wst[:D] + Wst[D:]) @ Wo) — so both
    output projections fold into one 1024x1024 matrix Weff.
  * softmax scale (0.125) is folded into K on the host; the padding-mask
    bias is a per-token column vector applied inside the exp activation
    (ACT computes func(in + bias), bias indexed by partition = token), so
    masking is free and there is a single code path.  No max-subtraction:
    scores are O(1) so exp cannot overflow.

Softmax bookkeeping (vs the 590us baseline):
  * Per-head softmax sums are collected into ONE PSUM tile via one-hot
    selector matmuls accumulated across the 16 heads of a rowtile.  This
    kills the per-head [1,512] ln/exp ACT chain of the baseline (~175us
    of ACT busy + the serial dependency that stalled the PE); reciprocals
    are then one ln + one negated-exp ACT op per rowtile.
  * The reciprocal is broadcast to A^T's 128 partitions with ONE selector
    matmul per head-PAIR, halving the baseline's rank-1 broadcast count.
  * PV results (A^T unnormalized, one [128, 512] tile per head pair) are
    copied PSUM->SBUF fp32 right after the pair completes so PSUM stays
    within 8 banks; the tail does bcast-matmul + DVE multiply into the
    packed bf16 A^T.

PE efficiency (the big one, ~65us): every attention matmul (qk, PV,
sums, bcast) uses weights zero-padded to a full 128x128 block.  Partial
tile_size matmuls cannot pipeline back-to-back on the PE array and pay
the full ~219-cycle fill latency per instruction (~305-340ns observed);
full blocks issue at stream rate (~225ns).  The zero weight rows also
nullify garbage in the padded rhs partitions -- provided that garbage is
finite, hence the full-height exp (pad scores are exact zeros ->
exp(0)=1) and the one-time zeroing of the reciprocal pool buffers.
The previous rowtile's output projection is interleaved between heads in
4-matmul batches to cover ACT/DVE latencies.

Sharding: pure data-parallel over (B, T_frames): 32 frames / 8 cores =
4 frames (4096 query rows) per core; K/V/weights replicated.  No
collectives.

Device layout is "transposed activations": X^T, Q^T, A^T all live as
[feature-part, row-free] tiles so every matmul is a natural slice.  Head h
occupies partitions (h%2)*64..+64 of feature chunk h//2; K^T is replicated
on both partition halves so odd heads read lane-aligned operands, and odd
heads' PV output is placed at PSUM base 64 (tile_position) so A^T lands on
partitions 64..127 without any cross-partition copies.
"""

import sys

if "/opt/trn_rl_repo" not in sys.path:
    sys.path.insert(0, "/opt/trn_rl_repo")

from contextlib import ExitStack

import ml_dtypes
import numpy as np

import concourse.bass as bass
import concourse.mybir as mybir
import concourse.tile as tile
from concourse import bacc
from concourse.bass_utils import run_bass_kernel_spmd

BF16 = ml_dtypes.bfloat16

D = 1024           # d_model
H = 16             # num heads
G = 4              # query groups
HD = 64            # head dim
HPG = H // G       # heads per group
SCALE = 0.125
B, T, HW, TT = 2, 16, 1024, 77
NCORES = 8
FPC = (B * T) // NCORES      # frames per core = 4
ROWS = FPC * HW              # 4096 query rows per core
RT = 512                     # rows per row-tile
NRT = ROWS // RT             # 8
NK = D // 128                # 8 partition chunks of d_model

_PROG_CACHE = {}


def _patch_act_tables():
    """Force every activation onto the one table set that contains Exp, Ln
    and Copy together (natural_log_exp_and_others, same 400-interval
    precision).  Without this, bacc's table-load pass can alternate between
    table sets, costing a ~1.28us ACT_TABLE_LOAD per switch."""
    import concourse.bacc as _bm
    import concourse.hw_specs as _hw
    if getattr(_bm, "_act_tables_patched", False):
        return
    _orig = _hw.get_activation_tables

    def patched(arch):
        t = dict(_orig(arch))
        combo = None
        for name, funcs in t.items():
            if (mybir.ActivationFunctionType.Exp in funcs
                    and mybir.ActivationFunctionType.Ln in funcs):
                combo = name
                break
        if combo is not None:
            for name in list(t):
                if name != combo:
                    t[name] = set()
        return t

    _bm.get_activation_tables = patched
    _bm._act_tables_patched = True

# test.py can flip these for profiling runs
TRACE = False
TRACE_KWARGS = {}
LAST_RESULTS = None


def _build_program():
    _patch_act_tables()
    dt = mybir.dt
    nc = bacc.Bacc("TRN2", target_bir_lowering=False, debug=False,
                   num_devices=NCORES)

    xt = nc.dram_tensor("xt", [D, ROWS], dt.bfloat16, kind="ExternalInput").ap()
    wq = nc.dram_tensor("wq", [D, D], dt.bfloat16, kind="ExternalInput").ap()
    weff = nc.dram_tensor("weff", [D, D], dt.bfloat16, kind="ExternalInput").ap()
    # All attention weights are zero-padded to full 128x128 blocks: partial
    # tile_size matmuls (64/77/16-row contract) cannot pipeline back-to-back
    # on the PE and pay the full ~219-cycle array latency each; full blocks
    # issue at stream rate.  Zero weight rows also nullify whatever garbage
    # sits in the padded partitions of the rhs operands, so no explicit
    # zeroing is needed anywhere.
    # per-head qk blocks: [128, H*128], block h rows (h%2)*64..+64 = K_g^T
    kt = nc.dram_tensor("kt", [128, H * 128], dt.bfloat16, kind="ExternalInput").ap()
    # per-head PV blocks: [128, H*128], block h rows 0..76 = V_g at cols (h%2)*64..+64
    v64 = nc.dram_tensor("v64", [128, H * 128], dt.bfloat16, kind="ExternalInput").ap()
    # per-head one-hot sum selectors: [128, H*128], block h: col h = 1 on rows 0..76
    sel = nc.dram_tensor("sel", [128, H * 128], dt.bfloat16, kind="ExternalInput").ap()
    # per-pair broadcast selectors: [128, 8*128], pair c: row 2c -> cols 0..63,
    # row 2c+1 -> cols 64..127
    selp = nc.dram_tensor("selp", [128, 8 * 128], dt.bfloat16, kind="ExternalInput").ap()
    # padding-mask bias as a per-partition column vector, applied inside the
    # exp activation (func(scale*in + bias)); all-zero when the mask is all-True
    biasr = nc.dram_tensor("biasr", [128, 1], dt.float32, kind="ExternalInput").ap()
    out = nc.dram_tensor("out", [ROWS, D], dt.float32, kind="ExternalOutput").ap()

    with tile.TileContext(nc) as tc, ExitStack() as ctx:
        wpool = ctx.enter_context(tc.tile_pool(name="weights", bufs=1))
        xpool = ctx.enter_context(tc.tile_pool(name="xt", bufs=3))
        x0pool = ctx.enter_context(tc.tile_pool(name="xt0", bufs=1))
        qpool = ctx.enter_context(tc.tile_pool(name="qt", bufs=2))
        apool = ctx.enter_context(tc.tile_pool(name="at", bufs=2))
        ppool = ctx.enter_context(tc.tile_pool(name="pt", bufs=4))
        aupool = ctx.enter_context(tc.tile_pool(name="au", bufs=9))
        rpool = ctx.enter_context(tc.tile_pool(name="recip", bufs=2))
        rbpool = ctx.enter_context(tc.tile_pool(name="recipb", bufs=2))
        opool = ctx.enter_context(tc.tile_pool(name="osb", bufs=3))
        # 8 PSUM banks: qp(2) + sprb(2, scores+bcast) + ap(2) + sums(1) + op(1)
        qpsum = ctx.enter_context(tc.tile_pool(name="qpsum", bufs=2, space="PSUM"))
        spsum = ctx.enter_context(tc.tile_pool(name="spsum", bufs=2, space="PSUM"))
        apsum = ctx.enter_context(tc.tile_pool(name="apsum", bufs=2, space="PSUM"))
        supsum = ctx.enter_context(tc.tile_pool(name="supsum", bufs=1, space="PSUM"))
        opsum = ctx.enter_context(tc.tile_pool(name="opsum", bufs=1, space="PSUM"))

        # --- resident weights; first rowtile's X^T + Wq lead the DMA queue so
        # the first Qproj chain starts ~8us earlier.  Wq lives as one tile
        # per chain (host repacked chain-major) and rowtile 0's X^T as one
        # tile per contract chunk, so each Qproj matmul depends only on its
        # own 0.25MB/0.125MB DMA and the PE starts ~2us in, pacing with DMA
        # arrival instead of waiting for the full 3MB.
        wqc = [wpool.tile([128, NK * 128], dt.bfloat16, tag=f"wq{cc}", name=f"wq{cc}")
               for cc in range(NK)]
        weff_t = wpool.tile([128, NK * D], dt.bfloat16, tag="weff")
        # startup is DMA-bound: spread the critical first loads (X^T rowtile 0
        # + Wq, 3MB total) across all three DMA trigger queues (SP, ACT,
        # GPSIMD) so they transfer in parallel; full-chunk shapes only (column
        # splits of these DMAs measured ~20% slower overall)
        # chunked startup DMAs spread across engines (a single merged DMA
        # runs on one engine and takes ~2x longer to land); merged 3D-AP
        # DMAs are used for the non-latency-critical loads below
        xt_r = xt.rearrange("(kc p) r -> p kc r", p=128)
        xt0c = [x0pool.tile([128, RT], dt.bfloat16, tag=f"x0{kc}", name=f"x0{kc}")
                for kc in range(NK)]
        for kc in range(NK):
            nc.scalar.dma_start(out=xt0c[kc][:],
                                in_=xt[kc * 128:(kc + 1) * 128, 0:RT])
            nc.sync.dma_start(out=wqc[kc][:],
                              in_=wq[kc * 128:(kc + 1) * 128, :])
        kt_t = wpool.tile([128, H * 128], dt.bfloat16, tag="kt")
        nc.sync.dma_start(out=kt_t[:], in_=kt[:, :])
        v64_t = wpool.tile([128, H * 128], dt.bfloat16, tag="v64")
        nc.sync.dma_start(out=v64_t[:], in_=v64[:, :])
        sel_t = wpool.tile([128, H * 128], dt.bfloat16, tag="sel")
        nc.sync.dma_start(out=sel_t[:], in_=sel[:, :])
        selp_t = wpool.tile([128, 8 * 128], dt.bfloat16, tag="selp")
        nc.sync.dma_start(out=selp_t[:], in_=selp[:, :])
        bias_t = wpool.tile([128, 1], dt.float32, tag="bias")
        nc.sync.dma_start(out=bias_t[:], in_=biasr[:, :])
        nc.sync.dma_start(
            out=weff_t[:, :].rearrange("p (kc c) -> p kc c", c=D),
            in_=weff.rearrange("(kc p) c -> p kc c", p=128))

        # one-time zeroing of the reciprocal tiles' pad rows: the tail's
        # broadcast matmul streams rcb[16:128] into zero weight rows, and
        # those bytes are never written by the [0:16] ACT ops, so clearing
        # the pool's physical buffers once keeps them finite forever
        for _ in range(2):
            rcz = rbpool.tile([128, RT], dt.bfloat16, tag="rcb")
            nc.vector.memset(rcz[:], 0.0)

        prev = None
        for rt in range(NRT):
            rsl = bass.ts(rt, RT)  # row slice in DRAM

            # --- load X^T row-tile: [1024 din, 512 rows] packed [128, 8*512]
            if rt > 0:
                xt_t = xpool.tile([128, NK * RT], dt.bfloat16, tag="xt")
                nc.sync.dma_start(
                    out=xt_t[:, :].rearrange("p (kc r) -> p kc r", r=RT),
                    in_=xt_r[:, :, rt * RT:(rt + 1) * RT],
                )

            # --- Q^T = Wq^T @ X^T : [1024 qcols, 512 rows] packed [128, 8*512]
            qt_t = qpool.tile([128, NK * RT], dt.bfloat16, tag="qt")
            for cc in range(NK):
                qp = qpsum.tile([128, RT], dt.float32, tag="qp")
                for kc in range(NK):
                    nc.tensor.matmul(
                        qp[:],
                        lhsT=wqc[cc][:, kc * 128:(kc + 1) * 128],
                        rhs=(xt0c[kc][:] if rt == 0
                             else xt_t[:, kc * RT:(kc + 1) * RT]),
                        start=(kc == 0), stop=(kc == NK - 1),
                    )
                nc.vector.tensor_copy(qt_t[:, cc * RT:(cc + 1) * RT], qp[:])

            # --- attention per head -> A^T packed [128, 8*512] (bf16)
            at_t = apool.tile([128, NK * RT], dt.bfloat16, tag="at")

            def wproj_gen(prt, pat, split_dma=False):
                # previous rowtile's Out projection as a stream of small
                # PE batches, pulled between attention heads so the PE has
                # dense independent work during ACT/DVE latencies.  PSUM
                # double-buffers by alternating the op pool with the (idle
                # during the head loop) Qproj pool.
                for rc4 in range(4):
                    ot = opool.tile([128, D], dt.float32, tag="ot")
                    for oc in range(2):
                        pool = opsum if (rc4 * 2 + oc) % 2 == 0 else qpsum
                        op_ = pool.tile([128, RT], dt.float32,
                                        tag="op" if pool is opsum else "qp")
                        for ac in range(NK):
                            nc.tensor.matmul(
                                op_[:],
                                lhsT=pat[:, ac * RT + rc4 * 128: ac * RT + (rc4 + 1) * 128],
                                rhs=weff_t[:, ac * D + oc * 512: ac * D + (oc + 1) * 512],
                                start=(ac == 0), stop=(ac == NK - 1),
                            )
                            if ac % 4 == 3:
                                yield
                        nc.vector.tensor_copy(ot[:, oc * 512:(oc + 1) * 512], op_[:])
                        if split_dma:
                            # final drain: store each half as soon as its copy
                            # lands so the last DMA overlaps the last PE chain
                            nc.sync.dma_start(
                                out=out[prt * RT + rc4 * 128: prt * RT + (rc4 + 1) * 128,
                                        oc * 512:(oc + 1) * 512],
                                in_=ot[:, oc * 512:(oc + 1) * 512],
                            )
                    if not split_dma:
                        nc.sync.dma_start(
                            out=out[prt * RT + rc4 * 128: prt * RT + (rc4 + 1) * 128, :],
                            in_=ot[:],
                        )

            wops = wproj_gen(prev[0], prev[1]) if prev is not None else None

            _done = object()

            def drain(k):
                if wops is None:
                    return
                for _ in range(k):
                    if next(wops, _done) is _done:
                        break

            sums = supsum.tile([128, RT], dt.float32, tag="sums")
            au_list = []
            pair_ap = [None]

            def head_front(h):
                csl = bass.ds((h // 2) * RT, RT)
                sp = spsum.tile([128, RT], dt.float32, tag="sprb")
                nc.tensor.matmul(
                    sp[:],
                    lhsT=kt_t[:, h * 128:(h + 1) * 128],
                    rhs=qt_t[:, csl],
                    start=True, stop=True,
                )
                # full-height exp: pad score rows are exact zeros from the
                # zero-padded qk weights, so pt rows 77..127 become exp(0)=1,
                # which the zero-padded sel/vp weight rows then ignore (a
                # finite value is required: 0-weight x inf/NaN garbage = NaN)
                pt = ppool.tile([128, RT], dt.bfloat16, tag="pt")
                nc.scalar.activation(pt[:], sp[:],
                                     mybir.ActivationFunctionType.Exp,
                                     bias=bias_t[:])
                return pt

            def head_back(h, pt):
                po = (h % 2) * HD
                # head h's softmax sum -> row h of the shared [128, RT] tile
                # (rows 16..127 accumulate zeros from the padded selector)
                nc.tensor.matmul(
                    sums[:],
                    lhsT=sel_t[:, h * 128:(h + 1) * 128],
                    rhs=pt[:],
                    start=(h == 0), stop=(h == H - 1),
                )
                if po == 0:
                    ap_ = apsum.tile([128, RT], dt.float32, tag="ap")
                    pair_ap[0] = ap_
                nc.tensor.matmul(
                    pair_ap[0][:],
                    lhsT=v64_t[:, h * 128:(h + 1) * 128],
                    rhs=pt[:],
                    start=(po == 0), stop=(po != 0),
                )
                if po != 0:
                    # pair complete: stash unnormalized A^T pair in SBUF fp32
                    au = aupool.tile([128, RT], dt.float32, tag="au")
                    nc.vector.tensor_copy(au[:], pair_ap[0][:])
                    au_list.append(au)

            # software-pipelined head loop: exp(h) overlaps qk(h+1) and the
            # previous head's sums/PV plus a Wproj batch
            prev_pt = None
            for h in range(H):
                pt = head_front(h)
                if prev_pt is not None:
                    head_back(h - 1, prev_pt)
                prev_pt = pt
                if h >= 2:
                    drain(1)
            head_back(H - 1, prev_pt)

            # --- tail: one reciprocal for all 16 heads, then per-pair
            # broadcast + normalize into packed A^T
            rcf = rpool.tile([16, RT], dt.float32, tag="rcf")
            nc.scalar.activation(rcf[0:16, :], sums[0:16, :],
                                 mybir.ActivationFunctionType.Ln)
            rcb = rbpool.tile([128, RT], dt.bfloat16, tag="rcb")
            nc.scalar.activation(rcb[0:16, :], rcf[0:16, :],
                                 mybir.ActivationFunctionType.Exp,
                                 scale=-1.0)
            drain(2)
            for c in range(NK):
                rb = spsum.tile([128, RT], dt.float32, tag="sprb")
                nc.tensor.matmul(
                    rb[:],
                    lhsT=selp_t[:, c * 128:(c + 1) * 128],
                    rhs=rcb[:],
                    start=True, stop=True,
                )
                nc.vector.tensor_mul(
                    at_t[:, c * RT:(c + 1) * RT], au_list[c][:], rb[:],
                )
                drain(1)
            drain(99)

            prev = (rt, at_t)

        # drain: Wproj of the final rowtile
        for _ in wproj_gen(prev[0], prev[1], split_dma=True):
            pass

    nc.compile()
    return nc


def _get_program():
    if "p" not in _PROG_CACHE:
        _PROG_CACHE["p"] = _build_program()
    return _PROG_CACHE["p"]


def _prep_inputs(x, te, mask, Wq, Wk, Wv, Wo, Wst):
    """Host-side fp32 weight prep + per-core shard maps."""
    K = (te @ Wk).reshape(B, TT, G, HD) * SCALE
    V = (te @ Wv).reshape(B, TT, G, HD)
    Weff = ((Wst[:D] + Wst[D:]) @ Wo).astype(np.float32)

    # chain-major repack: wqh[cc*128+p, kc*128+c] = Wq[kc*128+p, cc*128+c]
    wq_b = np.ascontiguousarray(
        Wq.reshape(8, 128, 8, 128).transpose(2, 1, 0, 3).reshape(D, D)
    ).astype(BF16)
    weff_b = Weff.astype(BF16)

    # per-head one-hot sum selectors [128, H*128]: block h col h = 1 on rows 0..76
    sel_np = np.zeros((128, H * 128), np.float32)
    for h in range(H):
        sel_np[0:TT, h * 128 + h] = 1.0
    sel_b = sel_np.astype(BF16)
    # per-pair broadcast selectors [128, 8*128]
    selp_np = np.zeros((128, 8 * 128), np.float32)
    for c in range(8):
        selp_np[2 * c, c * 128: c * 128 + 64] = 1.0
        selp_np[2 * c + 1, c * 128 + 64: c * 128 + 128] = 1.0
    selp_b = selp_np.astype(BF16)

    kt_b, v_b, bias_b = [], [], []
    for b in range(B):
        # qk blocks [128, H*128]: block h rows (h%2)*64..+64, cols 0..76 = K_g^T
        ktq = np.zeros((128, H * 128), np.float32)
        # PV blocks [128, H*128]: block h rows 0..76, cols (h%2)*64..+64 = V_g
        vp = np.zeros((128, H * 128), np.float32)
        for h in range(H):
            g = h // HPG
            po = (h % 2) * HD
            ktq[po:po + HD, h * 128:h * 128 + TT] = K[b, :, g, :].T
            vp[0:TT, h * 128 + po:h * 128 + po + HD] = V[b, :, g, :]
        kt_b.append(ktq.astype(BF16))
        v_b.append(vp.astype(BF16))
        bv = np.zeros((128, 1), np.float32)
        bv[0:TT, 0] = np.where(mask[b], 0.0, -30.0)
        bias_b.append(bv)

    in_maps = []
    for c in range(NCORES):
        b = c // (NCORES // B)
        fr = (c % (NCORES // B)) * FPC
        xc = x[b, fr:fr + FPC].reshape(ROWS, D).astype(BF16)
        m = {
            "xt": np.ascontiguousarray(xc.T),
            "wq": wq_b,
            "weff": weff_b,
            "kt": kt_b[b],
            "v64": v_b[b],
            "sel": sel_b,
            "selp": selp_b,
            "biasr": bias_b[b],
        }
        in_maps.append(m)
    return in_maps


def kernel(x, text_embeddings, padding_mask, use_mqa=0, use_qk_norm=0,
           Wq=None, Wk=None, Wv=None, Wo=None, Wst=None):
    global LAST_RESULTS
    x = np.asarray(x, np.float32)
    te = np.asarray(text_embeddings, np.float32)
    mask = np.asarray(padding_mask).astype(bool)
    Wq = np.asarray(Wq, np.float32)
    Wk = np.asarray(Wk, np.float32)
    Wv = np.asarray(Wv, np.float32)
    Wo = np.asarray(Wo, np.float32)
    Wst = np.asarray(Wst, np.float32)
    assert x.shape == (B, T, HW, D) and te.shape == (B, TT, D)

    in_maps = _prep_inputs(x, te, mask, Wq, Wk, Wv, Wo, Wst)
    nc = _get_program()

    res = run_bass_kernel_spmd(nc, in_maps, list(range(NCORES)),
                               trace=TRACE, **TRACE_KWARGS)
    LAST_RESULTS = res

    outp = np.empty((B, T, HW, D), np.float32)
    for c in range(NCORES):
        b = c // (NCORES // B)
        fr = (c % (NCORES // B)) * FPC
        outp[b, fr:fr + FPC] = res.results[c]["out"].reshape(FPC, HW, D)
    return outp
